# revision 18
# baseline (speedup 1.0000x reference)
"""CrossAttention kernel for 8 TRN2 NeuronCores.

Sharding: 8 cores = 4 batches x 2 query-halves (zero communication).
Each core computes all 16 heads for its 1024 queries.

v8 structure:
- AV computed in [q, d] orientation (lhsT = exp-scores tile, rhs = V):
  66.5k streamed columns instead of 131k for the [d, q] orientation.
  A ones-column appended to V gives the softmax denominator as column 64
  of each AV psum region -- no separate denominator matmuls.
- PSUM accumulations within one bank must be sequential (interleaving
  corrupts earlier regions), so heads are processed one at a time: head
  h's scores+exp stream in block h while head h-1's AV regions run
  region-major (kb innermost), packed 7-per-bank into 2 rolling psum
  banks one block behind.
- scores psum pool is 3-deep so the scores->exp->free chain never
  throttles the slot cadence; projections run as compact units through
  the same rotation.
- normalization is a per-partition DVE reciprocal + tensor_scalar
  multiply (q on partitions); normalized [q, 128] pair tiles go back to
  [inner, q] via DMA xbar transposes (zero PE cost), issue alternating
  between the vector and sync queues.
- input loads are tiered: critical path (cT, xT, wq0) serialized on the
  sync ring, wv on the scalar ring, bulk wk/wq chunks on the gpsimd
  ring behind a gate op that waits for cT so they cannot steal DMA
  bandwidth from the critical path.
"""

import sys

for _p in ("/opt/trn_rl_repo", "/root/.axon_site/_ro/trn_rl_repo"):
    if _p not in sys.path:
        sys.path.append(_p)

import numpy as np

import concourse.bass as bass
import concourse.tile as tile
from concourse import bacc, mybir
from concourse.bass_utils import run_bass_kernel_spmd

F32 = mybir.dt.float32
BF16 = mybir.dt.bfloat16
EXP = mybir.ActivationFunctionType.Exp
ADD = mybir.AluOpType.add

P = 128
B, NQ_FULL, DQ = 4, 2048, 1024
NK, DC = 1024, 768
H, DH = 16, 64
INNER = H * DH  # 1024
NT = 1024  # local queries per core
N_CORES = 8

KQ = DQ // P      # 8
KC = DC // P      # 6
KI = INNER // P   # 8
TB = NT // P      # 8 query tiles
KB = NK // P      # 8 kpos chunks
QB = NT // P      # 8 q-blocks for AV
HP = H // 2       # 8 head pairs
SCALE = 1.0 / np.sqrt(DH)


def build(dbg=False):
    nc = bacc.Bacc("TRN2", target_bir_lowering=False, debug=False,
                   enable_asserts=False, num_devices=N_CORES)

    cT_d = nc.dram_tensor("cT", [DC, NK], BF16, kind="ExternalInput")
    xpk_d = nc.dram_tensor("xpk", [P, 2, KQ, 512], BF16, kind="ExternalInput")
    wqpk_d = nc.dram_tensor("wqpk", [P, KI, KQ, P], BF16,
                            kind="ExternalInput")
    wkpk_d = nc.dram_tensor("wkpk", [P, KI, KC, P], BF16,
                            kind="ExternalInput")
    wvpk_d = nc.dram_tensor("wvpk", [P, 2, KC, 512], BF16,
                            kind="ExternalInput")
    wo_d = nc.dram_tensor("wo", [INNER, DQ], BF16, kind="ExternalInput")
    bo_d = nc.dram_tensor("bo", [DQ], BF16, kind="ExternalInput")
    out_d = nc.dram_tensor("out", [NT, DQ], BF16, kind="ExternalOutput")
    if dbg:
        dqT = nc.dram_tensor("dqT", [P, KI, NT], F32, kind="ExternalOutput")
        dkT = nc.dram_tensor("dkT", [P, KI, NK], F32, kind="ExternalOutput")
        dvA = nc.dram_tensor("dvA", [P, KB, H, DH + 1], F32,
                             kind="ExternalOutput")
        dattnT = nc.dram_tensor("dattnT", [P, KI, NT], F32,
                                kind="ExternalOutput")

    with tile.TileContext(nc) as tc:
        with (
            tc.tile_pool(name="persist", bufs=1) as persist,
            tc.tile_pool(name="psS", bufs=3, space="PSUM") as psS,
            tc.tile_pool(name="psAV", bufs=2, space="PSUM") as psAV,
            tc.tile_pool(name="etp", bufs=18) as etp,
            tc.tile_pool(name="aqp", bufs=20) as aqp,
            tc.tile_pool(name="recp", bufs=8) as recp,
            tc.tile_pool(name="outp", bufs=2) as outp,
        ):
            # persistent SBUF tensors
            cT = persist.tile([P, KC, NK], BF16)          # [dc, kpos]
            xT = persist.tile([P, 2, KQ, 512], BF16)      # [dq, (half,kc,q)]
            wq_b = persist.tile([P, KI, KQ, P], BF16)
            wk_b = persist.tile([P, KI, KC, P], BF16)
            wv_b = persist.tile([P, 2, KC, 512], BF16)
            wo_b = persist.tile([P, KI, DQ], BF16)
            bo_sb = persist.tile([1, DQ], BF16)
            ones_b = persist.tile([1, P], BF16)
            bias_b = persist.tile([P, DQ], BF16)          # bo bcast over parts
            qT = persist.tile([P, KI, NT], BF16)          # [inner, q]
            kT = persist.tile([P, KI, NK], BF16)          # [inner, kpos]
            vA = persist.tile([P, KB, H, DH + 1], BF16)   # [kpos,(h, d|1)]
            attnT = persist.tile([P, KI, NT], BF16)       # normalized attn^T

            # ---------------- input loads (tiered, consumer order) ------
            # The scheduler keeps emission order among ready DMAs per queue,
            # so the critical path (cT -> xT/wq0) leads all three DMA-capable
            # rings; bulk wk/wq chunks trail on the gpsimd ring.
            cT3 = cT_d.ap().rearrange("(o p) m -> p o m", p=P)
            wo4 = wo_d.ap().rearrange("(o p) m -> p o m", p=P)
            nc.sync.dma_start(bo_sb[:], bo_d.ap()[None, :])
            nc.sync.dma_start(cT[:, 0:2], cT3[:, 0:2])
            nc.sync.dma_start(xT[:, 0, 0:4], xpk_d.ap()[:, 0, 0:4])
            nc.scalar.dma_start(cT[:, 2:4], cT3[:, 2:4])
            nc.scalar.dma_start(xT[:, 0, 4:8], xpk_d.ap()[:, 0, 4:8])
            nc.scalar.dma_start(xT[:, 1, 4:8], xpk_d.ap()[:, 1, 4:8])
            nc.scalar.dma_start(wv_b[:, 0], wvpk_d.ap()[:, 0])
            nc.scalar.dma_start(wv_b[:, 1], wvpk_d.ap()[:, 1])
            nc.gpsimd.memset(vA[:, :, :, DH:DH + 1], 1.0)
            nc.gpsimd.memset(ones_b[:], 1.0)
            nc.gpsimd.dma_start(wk_b[:, 0], wkpk_d.ap()[:, 0])
            nc.gpsimd.dma_start(cT[:, 4:6], cT3[:, 4:6])
            nc.gpsimd.dma_start(wq_b[:, 0], wqpk_d.ap()[:, 0])
            nc.gpsimd.dma_start(xT[:, 1, 0:4], xpk_d.ap()[:, 1, 0:4])
            nc.gpsimd.dma_start(wk_b[:, 1], wkpk_d.ap()[:, 1])
            nc.gpsimd.dma_start(wk_b[:, 2], wkpk_d.ap()[:, 2])
            nc.gpsimd.dma_start(wq_b[:, 1], wqpk_d.ap()[:, 1])
            for ko in (3, 4, 5, 6, 7):
                nc.gpsimd.dma_start(wk_b[:, ko], wkpk_d.ap()[:, ko])
                nc.gpsimd.dma_start(wq_b[:, ko - 1], wqpk_d.ap()[:, ko - 1])
            nc.gpsimd.dma_start(wq_b[:, 7], wqpk_d.ap()[:, 7])
            nc.gpsimd.dma_start(wo_b[:], wo4)

            # ---------------- projection units (psS rotation) -----------
            def kproj(ko):
                ps = psS.tile([P, NT], F32, tag="big", name=f"kp{ko}")
                # ko=0 runs during the input loads: accumulate in the order
                # the cT thirds land (scalar, sync, gpsimd rings)
                kcs = (2, 3, 0, 1, 4, 5) if ko == 0 else tuple(range(KC))
                for n0 in (0, 512):
                    for i, kc in enumerate(kcs):
                        nc.tensor.matmul(
                            ps[:, n0:n0 + 512],
                            wk_b[:, ko, kc, :],
                            cT[:, kc, n0:n0 + 512],
                            start=(i == 0), stop=(i == KC - 1))
                nc.vector.tensor_copy(kT[:, ko, :], ps[:])

            def qproj(ko):
                ps = psS.tile([P, NT], F32, tag="big", name=f"qp{ko}")
                if ko == 0:
                    # bias_b broadcast rides in this psum tile first
                    for n0 in (0, 512):
                        nc.tensor.matmul(ps[:, n0:n0 + 512], ones_b[0:1, :],
                                         bo_sb[0:1, n0:n0 + 512],
                                         start=True, stop=True)
                    nc.vector.tensor_copy(bias_b[:], ps[:])
                for hf in (0, 1):
                    n0 = hf * 512
                    for kc in range(KQ):
                        nc.tensor.matmul(
                            ps[:, n0:n0 + 512],
                            wq_b[:, ko, kc, :],
                            xT[:, hf, kc, :],
                            start=(kc == 0), stop=(kc == KQ - 1))
                    nc.vector.tensor_copy(qT[:, ko, n0:n0 + 512],
                                          ps[:, n0:n0 + 512])

            def vproj(mt, half):
                ps = psS.tile([P, NT], F32, tag="big", name=f"vp{mt}_{half}")
                for kc in range(KC):
                    nc.tensor.matmul(
                        ps[:, 0:512],
                        cT[:, kc, mt * P:(mt + 1) * P],
                        wv_b[:, half, kc, :],
                        start=(kc == 0), stop=(kc == KC - 1))
                h0 = half * 8
                nc.vector.tensor_copy(
                    vA[:, mt, h0:h0 + 8, 0:DH],
                    ps[:, 0:512].rearrange("p (h d) -> p h d", d=DH))

            # ---------------- attention ----------------
            # AV regions packed 7-per-bank into rolling psum banks.
            av_banks = {}

            def av_region(g):
                b, off = divmod(g, 7)
                if b not in av_banks:
                    av_banks[b] = psAV.tile([P, 512], F32, tag="av",
                                            name=f"avb{b}")
                return av_banks[b][:, off * 65:off * 65 + 65]

            ets = {}       # (h, kb) -> exp tile
            aqs = {}       # (hp, qb) -> normalized pair tile
            pending_tp = []  # transposes deferred a block so sync never
                             # holds its SEQ waiting on fresh aq tiles

            def flush_tp(n=1):
                for _ in range(n):
                    if not pending_tp:
                        return
                    hp, qb = pending_tp.pop(0)
                    nc.sync.dma_start_transpose(
                        attnT[:, hp, qb * P:(qb + 1) * P],
                        aqs.pop((hp, qb))[:])

            def emit_av_region(h, qb):
                # region-major: one full kb accumulation, sequential in bank
                reg = av_region(h * QB + qb)
                for kb in range(KB):
                    nc.tensor.matmul(
                        reg,
                        ets[(h, kb)][:, qb * P:(qb + 1) * P],
                        vA[:, kb, h, :],
                        start=(kb == 0), stop=(kb == KB - 1))

            def emit_norm(h, qb):
                hp, hl = divmod(h, 2)
                reg = av_region(h * QB + qb)
                rec = recp.tile([P, 1], F32, tag="rec", name=f"rec{h}_{qb}")
                if hl == 0:
                    aqs[(hp, qb)] = aqp.tile([P, P], BF16, tag="aq",
                                             name=f"aq{hp}_{qb}")
                aq = aqs[(hp, qb)]
                nc.vector.reciprocal_approx_fast(rec[:], reg[:, DH:DH + 1])
                nc.vector.tensor_scalar_mul(
                    aq[:, hl * DH:(hl + 1) * DH], reg[:, 0:DH], rec[:])
                if hl == 1:
                    pending_tp.append((hp, qb))

            def head_block(h, extra):
                hp, hl = divmod(h, 2)
                base = hl * DH
                for kb in range(KB):
                    flush_tp(1)
                    ps = psS.tile([P, NT], F32, tag="big", name=f"sc{h}_{kb}")
                    for n0 in (0, 512):
                        nc.tensor.matmul(
                            ps[:, n0:n0 + 512],
                            kT[base:base + DH, hp, kb * P:(kb + 1) * P],
                            qT[base:base + DH, hp, n0:n0 + 512],
                            start=True, stop=True)
                    # cluster all AV regions in one stream to cut PE
                    # switches; region-major order within banks preserved
                    if h > 0 and kb == 2:
                        for qb in range(QB):
                            emit_av_region(h - 1, qb)
                    for fn in extra[kb]:
                        fn()
                    et = etp.tile([P, NT], BF16, tag="exp", name=f"et{h}_{kb}")
                    nc.scalar.activation(et[:], ps[:], EXP, scale=float(SCALE))
                    ets[(h, kb)] = et
                    if h > 0 and kb == 2:
                        for qb in range(QB):
                            emit_norm(h - 1, qb)

            # ---------------- out projection unit ------------------------
            out3 = out_d.ap().rearrange("(t p) d -> p t d", p=P)
            out_ps = {}

            def out_unit(mt, kcs, finish):
                if mt not in out_ps:
                    out_ps[mt] = psS.tile([P, NT], F32, tag="big",
                                          name=f"op{mt}")
                ps = out_ps[mt]
                # kc-outer; n0 banks see sequential accumulation streams
                for kc in kcs:
                    for n0 in (0, 512):
                        nc.tensor.matmul(
                            ps[:, n0:n0 + 512],
                            attnT[:, kc, mt * P:(mt + 1) * P],
                            wo_b[:, kc, n0:n0 + 512],
                            start=(kc == 0), stop=(finish and kc == KI - 1))
                if finish:
                    ot = outp.tile([P, DQ], BF16, tag="out", name=f"ot{mt}")
                    nc.vector.tensor_tensor(ot[:], ps[:], bias_b[:], ADD)
                    eng = nc.sync if mt % 2 == 0 else nc.scalar
                    eng.dma_start(out3[:, mt], ot[:])

            # ---------------- schedule ----------------
            # interleave the first k/q projections so the PE fills the
            # wait for the last cT third / xT halves with useful work
            kp0 = psS.tile([P, NT], F32, tag="big", name="kp0")
            qp0 = psS.tile([P, NT], F32, tag="big", name="qp0")
            for n0 in (0, 512):
                for i, kc in enumerate((2, 3, 0, 1)):
                    nc.tensor.matmul(kp0[:, n0:n0 + 512], wk_b[:, 0, kc, :],
                                     cT[:, kc, n0:n0 + 512],
                                     start=(i == 0), stop=False)
            bp0 = psS.tile([P, NT], F32, tag="big", name="bp0")
            for n0 in (0, 512):
                nc.tensor.matmul(bp0[:, n0:n0 + 512], ones_b[0:1, :],
                                 bo_sb[0:1, n0:n0 + 512],
                                 start=True, stop=True)
            nc.vector.tensor_copy(bias_b[:], bp0[:])
            for kc in range(KQ):
                nc.tensor.matmul(qp0[:, 0:512], wq_b[:, 0, kc, :],
                                 xT[:, 0, kc, :],
                                 start=(kc == 0), stop=(kc == KQ - 1))
            nc.vector.tensor_copy(qT[:, 0, 0:512], qp0[:, 0:512])
            for n0 in (0, 512):
                for i, kc in enumerate((4, 5)):
                    nc.tensor.matmul(kp0[:, n0:n0 + 512], wk_b[:, 0, kc, :],
                                     cT[:, kc, n0:n0 + 512],
                                     start=False, stop=(i == 1))
            nc.vector.tensor_copy(kT[:, 0, :], kp0[:])
            for kc in range(KQ):
                nc.tensor.matmul(qp0[:, 512:1024], wq_b[:, 0, kc, :],
                                 xT[:, 1, kc, :],
                                 start=(kc == 0), stop=(kc == KQ - 1))
            nc.vector.tensor_copy(qT[:, 0, 512:1024], qp0[:, 512:1024])
            for h in range(H):
                hp, hl = divmod(h, 2)
                extra = [[] for _ in range(KB)]
                if h == 0:
                    for mt in range(KB):
                        extra[mt].append(lambda mt=mt: vproj(mt, 0))
                    extra[2].append(lambda: kproj(1))
                if 1 <= h <= 8:
                    mt = h - 1
                    extra[1 if h < 8 else 4].append(
                        lambda mt=mt: vproj(mt, 1))
                # balanced projection placement: kproj on even blocks,
                # qproj on odd blocks, away from the slot-1 AV cluster
                if hl == 0 and 0 < hp < HP - 1:
                    extra[4].append(lambda ko=hp + 1: kproj(ko))
                if hl == 1 and hp < HP - 1:
                    extra[4].append(lambda ko=hp + 1: qproj(ko))
                if h == H - 1:
                    extra[3].append(lambda: out_unit(0, range(3), False))
                    extra[5].append(lambda: out_unit(0, range(3, KI - 1),
                                                     False))
                head_block(h, extra)

            # ---------------- tail: last head's AV + out projection -----
            for mt in range(TB):
                emit_av_region(H - 1, mt)
                emit_norm(H - 1, mt)
                flush_tp(2)
            flush_tp(len(pending_tp))
            out_unit(0, [KI - 1], True)
            for mt in range(1, TB):
                out_unit(mt, range(KI), True)

            if dbg:
                nc.gpsimd.dma_start(dqT.ap(), qT[:])
                nc.gpsimd.dma_start(dkT.ap(), kT[:])
                nc.gpsimd.dma_start(dvA.ap(), vA[:])
                nc.gpsimd.dma_start(dattnT.ap(), attnT[:])

    nc.compile()
    return nc


_NC_CACHE = None


def _make_in_maps(inputs):
    import ml_dtypes
    bf = ml_dtypes.bfloat16
    x = np.asarray(inputs["x"], dtype=np.float32).astype(bf)
    context = np.asarray(inputs["context"], dtype=np.float32).astype(bf)
    wq = np.asarray(inputs["Wq"], np.float32).astype(bf)
    wk = np.asarray(inputs["Wk"], np.float32).astype(bf)
    wv = np.asarray(inputs["Wv"], np.float32).astype(bf)
    shared = {
        # [dq_chunk p, ko, kc, j]: per-(p, ko) contiguous 1536/2048B runs
        "wqpk": np.ascontiguousarray(
            wq.reshape(KQ, P, KI, P).transpose(1, 2, 0, 3)),
        "wkpk": np.ascontiguousarray(
            wk.reshape(KC, P, KI, P).transpose(1, 2, 0, 3)),
        "wvpk": np.ascontiguousarray(
            wv.reshape(KC, P, 2, 512).transpose(1, 2, 0, 3)),
        "wo": np.ascontiguousarray(np.asarray(inputs["Wo"], np.float32)
                                   .astype(bf)),
        "bo": np.ascontiguousarray(np.asarray(inputs["bo"], np.float32)
                                   .astype(bf)),
    }
    in_maps = []
    for c in range(N_CORES):
        b, s = divmod(c, 2)
        xTh = np.ascontiguousarray(x[b, s * NT:(s + 1) * NT, :].T)  # [dq, q]
        in_maps.append({
            "xpk": np.ascontiguousarray(
                xTh.reshape(KQ, P, 2, 512).transpose(1, 2, 0, 3)),
            "cT": np.ascontiguousarray(context[b].T),
            **shared,
        })
    return in_maps


def kernel(x, context, Wq, Wk, Wv, Wo, bo):
    global _NC_CACHE
    if _NC_CACHE is None:
        _NC_CACHE = build()
    nc = _NC_CACHE

    in_maps = _make_in_maps(dict(x=x, context=context, Wq=Wq, Wk=Wk, Wv=Wv,
                                 Wo=Wo, bo=bo))
    res = run_bass_kernel_spmd(nc, in_maps, core_ids=list(range(N_CORES)))
    out = np.empty((B, NQ_FULL, DQ), dtype=np.float32)
    for c in range(N_CORES):
        b, s = divmod(c, 2)
        out[b, s * NT:(s + 1) * NT, :] = res.results[c]["out"].astype(
            np.float32)
    return out


# revision 19
# speedup vs baseline: 1.0002x; 1.0002x over previous
"""CrossAttention kernel for 8 TRN2 NeuronCores.

Sharding: 8 cores = 4 batches x 2 query-halves (zero communication).
Each core computes all 16 heads for its 1024 queries.

v8 structure:
- AV computed in [q, d] orientation (lhsT = exp-scores tile, rhs = V):
  66.5k streamed columns instead of 131k for the [d, q] orientation.
  A ones-column appended to V gives the softmax denominator as column 64
  of each AV psum region -- no separate denominator matmuls.
- PSUM accumulations within one bank must be sequential (interleaving
  corrupts earlier regions), so heads are processed one at a time: head
  h's scores+exp stream in block h while head h-1's AV regions run
  region-major (kb innermost), packed 7-per-bank into 2 rolling psum
  banks one block behind.
- scores psum pool is 3-deep so the scores->exp->free chain never
  throttles the slot cadence; projections run as compact units through
  the same rotation.
- normalization is a per-partition DVE reciprocal + tensor_scalar
  multiply (q on partitions); normalized [q, 128] pair tiles go back to
  [inner, q] via DMA xbar transposes (zero PE cost), issue alternating
  between the vector and sync queues.
- input loads are tiered: critical path (cT, xT, wq0) serialized on the
  sync ring, wv on the scalar ring, bulk wk/wq chunks on the gpsimd
  ring behind a gate op that waits for cT so they cannot steal DMA
  bandwidth from the critical path.
"""

import sys

for _p in ("/opt/trn_rl_repo", "/root/.axon_site/_ro/trn_rl_repo"):
    if _p not in sys.path:
        sys.path.append(_p)

import numpy as np

import concourse.bass as bass
import concourse.tile as tile
from concourse import bacc, mybir
from concourse.bass_utils import run_bass_kernel_spmd

F32 = mybir.dt.float32
BF16 = mybir.dt.bfloat16
EXP = mybir.ActivationFunctionType.Exp
ADD = mybir.AluOpType.add

P = 128
B, NQ_FULL, DQ = 4, 2048, 1024
NK, DC = 1024, 768
H, DH = 16, 64
INNER = H * DH  # 1024
NT = 1024  # local queries per core
N_CORES = 8

KQ = DQ // P      # 8
KC = DC // P      # 6
KI = INNER // P   # 8
TB = NT // P      # 8 query tiles
KB = NK // P      # 8 kpos chunks
QB = NT // P      # 8 q-blocks for AV
HP = H // 2       # 8 head pairs
SCALE = 1.0 / np.sqrt(DH)


def build(dbg=False):
    nc = bacc.Bacc("TRN2", target_bir_lowering=False, debug=False,
                   enable_asserts=False, num_devices=N_CORES)

    cT_d = nc.dram_tensor("cT", [DC, NK], BF16, kind="ExternalInput")
    xpk_d = nc.dram_tensor("xpk", [P, 2, KQ, 512], BF16, kind="ExternalInput")
    wqpk_d = nc.dram_tensor("wqpk", [P, KI, KQ, P], BF16,
                            kind="ExternalInput")
    wkpk_d = nc.dram_tensor("wkpk", [P, KI, KC, P], BF16,
                            kind="ExternalInput")
    wvpk_d = nc.dram_tensor("wvpk", [P, 2, KC, 512], BF16,
                            kind="ExternalInput")
    wo_d = nc.dram_tensor("wo", [INNER, DQ], BF16, kind="ExternalInput")
    bo_d = nc.dram_tensor("bo", [DQ], BF16, kind="ExternalInput")
    out_d = nc.dram_tensor("out", [NT, DQ], BF16, kind="ExternalOutput")
    if dbg:
        dqT = nc.dram_tensor("dqT", [P, KI, NT], F32, kind="ExternalOutput")
        dkT = nc.dram_tensor("dkT", [P, KI, NK], F32, kind="ExternalOutput")
        dvA = nc.dram_tensor("dvA", [P, KB, H, DH + 1], F32,
                             kind="ExternalOutput")
        dattnT = nc.dram_tensor("dattnT", [P, KI, NT], F32,
                                kind="ExternalOutput")

    with tile.TileContext(nc) as tc:
        with (
            tc.tile_pool(name="persist", bufs=1) as persist,
            tc.tile_pool(name="psS", bufs=3, space="PSUM") as psS,
            tc.tile_pool(name="psAV", bufs=2, space="PSUM") as psAV,
            tc.tile_pool(name="etp", bufs=18) as etp,
            tc.tile_pool(name="aqp", bufs=20) as aqp,
            tc.tile_pool(name="recp", bufs=8) as recp,
            tc.tile_pool(name="outp", bufs=2) as outp,
        ):
            # persistent SBUF tensors
            cT = persist.tile([P, KC, NK], BF16)          # [dc, kpos]
            xT = persist.tile([P, 2, KQ, 512], BF16)      # [dq, (half,kc,q)]
            wq_b = persist.tile([P, KI, KQ, P], BF16)
            wk_b = persist.tile([P, KI, KC, P], BF16)
            wv_b = persist.tile([P, 2, KC, 512], BF16)
            wo_b = persist.tile([P, KI, DQ], BF16)
            bo_sb = persist.tile([1, DQ], BF16)
            ones_b = persist.tile([1, P], BF16)
            bias_b = persist.tile([P, DQ], BF16)          # bo bcast over parts
            qT = persist.tile([P, KI, NT], BF16)          # [inner, q]
            kT = persist.tile([P, KI, NK], BF16)          # [inner, kpos]
            vA = persist.tile([P, KB, H, DH + 1], BF16)   # [kpos,(h, d|1)]
            attnT = persist.tile([P, KI, NT], BF16)       # normalized attn^T

            # ---------------- input loads (tiered, consumer order) ------
            # The scheduler keeps emission order among ready DMAs per queue,
            # so the critical path (cT -> xT/wq0) leads all three DMA-capable
            # rings; bulk wk/wq chunks trail on the gpsimd ring.
            cT3 = cT_d.ap().rearrange("(o p) m -> p o m", p=P)
            wo4 = wo_d.ap().rearrange("(o p) m -> p o m", p=P)
            nc.sync.dma_start(bo_sb[:], bo_d.ap()[None, :])
            nc.sync.dma_start(cT[:, 0:2], cT3[:, 0:2])
            nc.sync.dma_start(xT[:, 0, 0:4], xpk_d.ap()[:, 0, 0:4])
            nc.scalar.dma_start(cT[:, 2:4], cT3[:, 2:4])
            nc.scalar.dma_start(xT[:, 0, 4:8], xpk_d.ap()[:, 0, 4:8])
            nc.scalar.dma_start(xT[:, 1, 4:8], xpk_d.ap()[:, 1, 4:8])
            nc.scalar.dma_start(wv_b[:, 0], wvpk_d.ap()[:, 0])
            nc.scalar.dma_start(wv_b[:, 1], wvpk_d.ap()[:, 1])
            nc.gpsimd.memset(vA[:, :, :, DH:DH + 1], 1.0)
            nc.gpsimd.memset(ones_b[:], 1.0)
            nc.gpsimd.dma_start(wk_b[:, 0], wkpk_d.ap()[:, 0])
            nc.gpsimd.dma_start(cT[:, 4:6], cT3[:, 4:6])
            nc.gpsimd.dma_start(wq_b[:, 0], wqpk_d.ap()[:, 0])
            nc.gpsimd.dma_start(xT[:, 1, 0:4], xpk_d.ap()[:, 1, 0:4])
            nc.gpsimd.dma_start(wk_b[:, 1], wkpk_d.ap()[:, 1])
            nc.gpsimd.dma_start(wk_b[:, 2], wkpk_d.ap()[:, 2])
            nc.gpsimd.dma_start(wq_b[:, 1], wqpk_d.ap()[:, 1])
            for ko in (3, 4, 5, 6, 7):
                nc.gpsimd.dma_start(wk_b[:, ko], wkpk_d.ap()[:, ko])
                nc.gpsimd.dma_start(wq_b[:, ko - 1], wqpk_d.ap()[:, ko - 1])
            nc.gpsimd.dma_start(wq_b[:, 7], wqpk_d.ap()[:, 7])
            nc.gpsimd.dma_start(wo_b[:], wo4)

            # ---------------- projection units (psS rotation) -----------
            def kproj(ko):
                ps = psS.tile([P, NT], F32, tag="big", name=f"kp{ko}")
                # ko=0 runs during the input loads: accumulate in the order
                # the cT thirds land (scalar, sync, gpsimd rings)
                kcs = (2, 3, 0, 1, 4, 5) if ko == 0 else tuple(range(KC))
                for n0 in (0, 512):
                    for i, kc in enumerate(kcs):
                        nc.tensor.matmul(
                            ps[:, n0:n0 + 512],
                            wk_b[:, ko, kc, :],
                            cT[:, kc, n0:n0 + 512],
                            start=(i == 0), stop=(i == KC - 1))
                nc.vector.tensor_copy(kT[:, ko, :], ps[:])

            def qproj(ko):
                ps = psS.tile([P, NT], F32, tag="big", name=f"qp{ko}")
                if ko == 0:
                    # bias_b broadcast rides in this psum tile first
                    for n0 in (0, 512):
                        nc.tensor.matmul(ps[:, n0:n0 + 512], ones_b[0:1, :],
                                         bo_sb[0:1, n0:n0 + 512],
                                         start=True, stop=True)
                    nc.vector.tensor_copy(bias_b[:], ps[:])
                for hf in (0, 1):
                    n0 = hf * 512
                    for kc in range(KQ):
                        nc.tensor.matmul(
                            ps[:, n0:n0 + 512],
                            wq_b[:, ko, kc, :],
                            xT[:, hf, kc, :],
                            start=(kc == 0), stop=(kc == KQ - 1))
                    nc.vector.tensor_copy(qT[:, ko, n0:n0 + 512],
                                          ps[:, n0:n0 + 512])

            def vproj(mt, half):
                ps = psS.tile([P, NT], F32, tag="big", name=f"vp{mt}_{half}")
                for kc in range(KC):
                    nc.tensor.matmul(
                        ps[:, 0:512],
                        cT[:, kc, mt * P:(mt + 1) * P],
                        wv_b[:, half, kc, :],
                        start=(kc == 0), stop=(kc == KC - 1))
                h0 = half * 8
                nc.vector.tensor_copy(
                    vA[:, mt, h0:h0 + 8, 0:DH],
                    ps[:, 0:512].rearrange("p (h d) -> p h d", d=DH))

            # ---------------- attention ----------------
            # AV regions packed 7-per-bank into rolling psum banks.
            av_banks = {}

            def av_region(g):
                b, off = divmod(g, 7)
                if b not in av_banks:
                    av_banks[b] = psAV.tile([P, 512], F32, tag="av",
                                            name=f"avb{b}")
                return av_banks[b][:, off * 65:off * 65 + 65]

            ets = {}       # (h, kb) -> exp tile
            aqs = {}       # (hp, qb) -> normalized pair tile
            pending_tp = []  # transposes deferred a block so sync never
                             # holds its SEQ waiting on fresh aq tiles

            def flush_tp(n=1):
                for _ in range(n):
                    if not pending_tp:
                        return
                    hp, qb = pending_tp.pop(0)
                    nc.sync.dma_start_transpose(
                        attnT[:, hp, qb * P:(qb + 1) * P],
                        aqs.pop((hp, qb))[:])

            def emit_av_region(h, qb):
                # region-major: one full kb accumulation, sequential in bank
                reg = av_region(h * QB + qb)
                for kb in range(KB):
                    nc.tensor.matmul(
                        reg,
                        ets[(h, kb)][:, qb * P:(qb + 1) * P],
                        vA[:, kb, h, :],
                        start=(kb == 0), stop=(kb == KB - 1))

            def emit_norm(h, qb):
                hp, hl = divmod(h, 2)
                reg = av_region(h * QB + qb)
                rec = recp.tile([P, 1], F32, tag="rec", name=f"rec{h}_{qb}")
                if hl == 0:
                    aqs[(hp, qb)] = aqp.tile([P, P], BF16, tag="aq",
                                             name=f"aq{hp}_{qb}")
                aq = aqs[(hp, qb)]
                nc.vector.reciprocal_approx_fast(rec[:], reg[:, DH:DH + 1])
                nc.vector.tensor_scalar_mul(
                    aq[:, hl * DH:(hl + 1) * DH], reg[:, 0:DH], rec[:])
                if hl == 1:
                    pending_tp.append((hp, qb))

            def head_block(h, extra):
                hp, hl = divmod(h, 2)
                base = hl * DH
                for kb in range(KB):
                    flush_tp(1)
                    ps = psS.tile([P, NT], F32, tag="big", name=f"sc{h}_{kb}")
                    for n0 in (0, 512):
                        nc.tensor.matmul(
                            ps[:, n0:n0 + 512],
                            kT[base:base + DH, hp, kb * P:(kb + 1) * P],
                            qT[base:base + DH, hp, n0:n0 + 512],
                            start=True, stop=True)
                    # cluster all AV regions in one stream to cut PE
                    # switches; region-major order within banks preserved
                    if h > 0 and kb == 2:
                        for qb in range(QB):
                            emit_av_region(h - 1, qb)
                    for fn in extra[kb]:
                        fn()
                    et = etp.tile([P, NT], BF16, tag="exp", name=f"et{h}_{kb}")
                    nc.scalar.activation(et[:], ps[:], EXP, scale=float(SCALE))
                    ets[(h, kb)] = et
                    if h > 0 and kb == 2:
                        for qb in range(QB):
                            emit_norm(h - 1, qb)

            # ---------------- out projection unit ------------------------
            out3 = out_d.ap().rearrange("(t p) d -> p t d", p=P)
            out_ps = {}

            def out_unit(mt, kcs, finish):
                if mt not in out_ps:
                    out_ps[mt] = psS.tile([P, NT], F32, tag="big",
                                          name=f"op{mt}")
                ps = out_ps[mt]
                # kc-outer; n0 banks see sequential accumulation streams
                for kc in kcs:
                    for n0 in (0, 512):
                        nc.tensor.matmul(
                            ps[:, n0:n0 + 512],
                            attnT[:, kc, mt * P:(mt + 1) * P],
                            wo_b[:, kc, n0:n0 + 512],
                            start=(kc == 0), stop=(finish and kc == KI - 1))
                if finish:
                    ot = outp.tile([P, DQ], BF16, tag="out", name=f"ot{mt}")
                    nc.vector.tensor_tensor(ot[:], ps[:], bias_b[:], ADD)
                    eng = nc.sync if mt % 2 == 0 else nc.scalar
                    eng.dma_start(out3[:, mt], ot[:])

            # ---------------- schedule ----------------
            # interleave the first k/q projections so the PE fills the
            # wait for the last cT third / xT halves with useful work
            kp0 = psS.tile([P, NT], F32, tag="big", name="kp0")
            qp0 = psS.tile([P, NT], F32, tag="big", name="qp0")
            for n0 in (0, 512):
                for i, kc in enumerate((2, 3, 0, 1)):
                    nc.tensor.matmul(kp0[:, n0:n0 + 512], wk_b[:, 0, kc, :],
                                     cT[:, kc, n0:n0 + 512],
                                     start=(i == 0), stop=False)
            bp0 = psS.tile([P, NT], F32, tag="big", name="bp0")
            for n0 in (0, 512):
                nc.tensor.matmul(bp0[:, n0:n0 + 512], ones_b[0:1, :],
                                 bo_sb[0:1, n0:n0 + 512],
                                 start=True, stop=True)
            nc.vector.tensor_copy(bias_b[:], bp0[:])
            for kc in range(KQ):
                nc.tensor.matmul(qp0[:, 0:512], wq_b[:, 0, kc, :],
                                 xT[:, 0, kc, :],
                                 start=(kc == 0), stop=(kc == KQ - 1))
            nc.vector.tensor_copy(qT[:, 0, 0:512], qp0[:, 0:512])
            for n0 in (0, 512):
                for i, kc in enumerate((4, 5)):
                    nc.tensor.matmul(kp0[:, n0:n0 + 512], wk_b[:, 0, kc, :],
                                     cT[:, kc, n0:n0 + 512],
                                     start=False, stop=(i == 1))
            nc.vector.tensor_copy(kT[:, 0, :], kp0[:])
            for kc in range(KQ):
                nc.tensor.matmul(qp0[:, 512:1024], wq_b[:, 0, kc, :],
                                 xT[:, 1, kc, :],
                                 start=(kc == 0), stop=(kc == KQ - 1))
            nc.vector.tensor_copy(qT[:, 0, 512:1024], qp0[:, 512:1024])
            for h in range(H):
                hp, hl = divmod(h, 2)
                extra = [[] for _ in range(KB)]
                if h == 0:
                    for mt in range(KB):
                        extra[mt].append(lambda mt=mt: vproj(mt, 0))
                    extra[6].append(lambda: kproj(1))
                if 1 <= h <= 8:
                    mt = h - 1
                    extra[6 if h < 8 else 4].append(
                        lambda mt=mt: vproj(mt, 1))
                # balanced projection placement: kproj on even blocks,
                # qproj on odd blocks, away from the slot-1 AV cluster
                if hl == 0 and 0 < hp < HP - 1:
                    extra[4].append(lambda ko=hp + 1: kproj(ko))
                if hl == 1 and hp < HP - 1:
                    extra[4].append(lambda ko=hp + 1: qproj(ko))
                if h == H - 1:
                    extra[3].append(lambda: out_unit(0, range(KI - 1), False))
                head_block(h, extra)

            # ---------------- tail: last head's AV + out projection -----
            for mt in range(TB):
                emit_av_region(H - 1, mt)
                emit_norm(H - 1, mt)
                flush_tp(2)
            flush_tp(len(pending_tp))
            out_unit(0, [KI - 1], True)
            for mt in range(1, TB):
                out_unit(mt, range(KI), True)

            if dbg:
                nc.gpsimd.dma_start(dqT.ap(), qT[:])
                nc.gpsimd.dma_start(dkT.ap(), kT[:])
                nc.gpsimd.dma_start(dvA.ap(), vA[:])
                nc.gpsimd.dma_start(dattnT.ap(), attnT[:])

    nc.compile()
    return nc


_NC_CACHE = None


def _make_in_maps(inputs):
    import ml_dtypes
    bf = ml_dtypes.bfloat16
    x = np.asarray(inputs["x"], dtype=np.float32).astype(bf)
    context = np.asarray(inputs["context"], dtype=np.float32).astype(bf)
    wq = np.asarray(inputs["Wq"], np.float32).astype(bf)
    wk = np.asarray(inputs["Wk"], np.float32).astype(bf)
    wv = np.asarray(inputs["Wv"], np.float32).astype(bf)
    shared = {
        # [dq_chunk p, ko, kc, j]: per-(p, ko) contiguous 1536/2048B runs
        "wqpk": np.ascontiguousarray(
            wq.reshape(KQ, P, KI, P).transpose(1, 2, 0, 3)),
        "wkpk": np.ascontiguousarray(
            wk.reshape(KC, P, KI, P).transpose(1, 2, 0, 3)),
        "wvpk": np.ascontiguousarray(
            wv.reshape(KC, P, 2, 512).transpose(1, 2, 0, 3)),
        "wo": np.ascontiguousarray(np.asarray(inputs["Wo"], np.float32)
                                   .astype(bf)),
        "bo": np.ascontiguousarray(np.asarray(inputs["bo"], np.float32)
                                   .astype(bf)),
    }
    in_maps = []
    for c in range(N_CORES):
        b, s = divmod(c, 2)
        xTh = np.ascontiguousarray(x[b, s * NT:(s + 1) * NT, :].T)  # [dq, q]
        in_maps.append({
            "xpk": np.ascontiguousarray(
                xTh.reshape(KQ, P, 2, 512).transpose(1, 2, 0, 3)),
            "cT": np.ascontiguousarray(context[b].T),
            **shared,
        })
    return in_maps


def kernel(x, context, Wq, Wk, Wv, Wo, bo):
    global _NC_CACHE
    if _NC_CACHE is None:
        _NC_CACHE = build()
    nc = _NC_CACHE

    in_maps = _make_in_maps(dict(x=x, context=context, Wq=Wq, Wk=Wk, Wv=Wv,
                                 Wo=Wo, bo=bo))
    res = run_bass_kernel_spmd(nc, in_maps, core_ids=list(range(N_CORES)))
    out = np.empty((B, NQ_FULL, DQ), dtype=np.float32)
    for c in range(N_CORES):
        b, s = divmod(c, 2)
        out[b, s * NT:(s + 1) * NT, :] = res.results[c]["out"].astype(
            np.float32)
    return out
